# revision 8
# baseline (speedup 1.0000x reference)
"""Trainium2 Bass kernel for a single-head causal attention block.

Reference computation (per batch b):
    q = x @ Wq + bq ; k = x @ Wk + bk ; v = x @ Wv + bv          [T, H]
    wei = softmax(causal(q @ k.T * C**-0.5))                      [T, T]
    out = wei @ v                                                 [T, H]

Sharding over 8 NeuronCores: core = 2*b + half. Each core handles one
batch and half of the T query rows. Row blocks (128 rows each) are
assigned to halves in a balanced zig-zag pattern (0,3,4,7,... vs
1,2,5,6,...) so both halves see the same causal workload profile: the
j-th slot of either half processes a key range of exactly (2j+2)*128
columns, which keeps the on-device program identical across cores (pure
SPMD) — the gap between the slot's key range and the row block's true
causal boundary is handled by a small additive mask supplied as input
data. The host permutes the query rows per half so slot j's rows are
contiguous, and scatters the output rows back.

On-device plan (per core):
  phase A: transpose x -> xT (PE), project kT (f32r matmuls), project v
           (bias folded in) to a DRAM scratch in bf16
  phase B: transpose xq -> xqT, project qT
  phase C: per slot: wei = qT.T @ kT (bf16), mask+softmax (exp on ACT
           with fused row-sum), transpose P (PE), out = P.T @ v with
           late normalization fused into the PSUM->SBUF copy.
"""

import math

import numpy as np

P = 128
B, T, C, H = 4, 2048, 1024, 2048
NEG = -1.0e10


def half_blocks(nb: int) -> tuple[list[int], list[int]]:
    h0, h1 = [], []
    for g in range(nb // 4):
        h0 += [4 * g, 4 * g + 3]
        h1 += [4 * g + 1, 4 * g + 2]
    return h0, h1


def make_masks(t: int) -> np.ndarray:
    """masks[half][slot] : [P, 2P] additive mask for the last 2 chunks of
    the slot's key range."""
    nb = t // P
    masks = np.zeros((2, nb // 2, P, 2 * P), dtype=np.float32)
    for half, blocks in enumerate(half_blocks(nb)):
        for j, pb in enumerate(sorted(blocks)):
            s_end = (2 * j + 2) * P
            col = s_end - 2 * P + np.arange(2 * P)[None, :]
            trow = pb * P + np.arange(P)[:, None]
            masks[half, j] = np.where(col <= trow, 0.0, NEG)
    return masks


def build_nc(t: int = T, c: int = C, h: int = H):
    import concourse.bass as bass
    import concourse.mybir as mybir
    import concourse.tile as tile
    from concourse import bacc
    from concourse.masks import make_identity

    f32 = mybir.dt.float32
    f32r = mybir.dt.float32r
    bf16 = mybir.dt.bfloat16

    nb = t // P          # key/row blocks per batch
    ns = nb // 2         # slots (row blocks per core)
    ck = c // P          # contraction chunks
    hk = h // P          # head chunks
    tq = t // 2          # query rows per core
    hs = 256             # head columns per weight strip
    ts = min(512, t)     # t columns per projection matmul
    scale = float(c) ** -0.5

    nc = bacc.Bacc("TRN2", target_bir_lowering=False, debug=False, num_devices=8)

    xkv = nc.dram_tensor("xkv", [t, c], f32, kind="ExternalInput").ap()
    xq = nc.dram_tensor("xq", [tq, c], f32, kind="ExternalInput").ap()
    wq = nc.dram_tensor("wq", [c, h], f32, kind="ExternalInput").ap()
    wk = nc.dram_tensor("wk", [c, h], f32, kind="ExternalInput").ap()
    wv = nc.dram_tensor("wv", [c, h], f32, kind="ExternalInput").ap()
    bq = nc.dram_tensor("bq", [h], f32, kind="ExternalInput").ap()
    bk = nc.dram_tensor("bk", [h], f32, kind="ExternalInput").ap()
    bv = nc.dram_tensor("bv", [h], f32, kind="ExternalInput").ap()
    mask = nc.dram_tensor("mask", [ns, P, 2 * P], f32, kind="ExternalInput").ap()
    out = nc.dram_tensor("out", [tq, h], f32, kind="ExternalOutput").ap()

    with tile.TileContext(nc) as tc:
        with (
            tc.tile_pool(name="singles", bufs=1) as singles,
            tc.tile_pool(name="ktp", bufs=1) as ktp,
            tc.tile_pool(name="qtp", bufs=1) as qtp,
            tc.tile_pool(name="vdram", bufs=1, space="DRAM") as vdram,
        ):
            ident32 = singles.tile([P, P], f32)
            make_identity(nc, ident32)
            ident16 = singles.tile([P, P], bf16)
            make_identity(nc, ident16)
            bq_t = singles.tile([P, hk], f32)
            nc.sync.dma_start(out=bq_t, in_=bq.rearrange("(k p) -> p k", p=P))
            bk_t = singles.tile([P, hk], f32)
            nc.sync.dma_start(out=bk_t, in_=bk.rearrange("(k p) -> p k", p=P))

            kt = ktp.tile([P, hk, t], bf16)
            qt = qtp.tile([P, hk, tq], bf16)
            vs = vdram.tile([nb, P, h], bf16)

            # ---- phase A: xT, kT, v (T processed in halves to fit SBUF) ----
            tha = max(1, t // 1024)   # t-half passes
            thl = t // tha            # rows per pass
            with (
                tc.tile_pool(name="bvp", bufs=1) as bvp,
                tc.tile_pool(name="xtp", bufs=1) as xtp,
                tc.tile_pool(name="xsp", bufs=2) as xsp,
                tc.tile_pool(name="wkp", bufs=2) as wkp,
                tc.tile_pool(name="wvp", bufs=2) as wvp,
                tc.tile_pool(name="vstg", bufs=2) as vstg,
                tc.tile_pool(name="tp_ps", bufs=2, space="PSUM") as tp_ps,
                tc.tile_pool(name="pr_ps", bufs=3, space="PSUM") as pr_ps,
            ):
                bv_t = bvp.tile([P, h], f32)
                bv_bcast = bass.AP(tensor=bv.tensor, offset=bv.offset,
                                   ap=[[0, P], [1, h]])
                nc.sync.dma_start(out=bv_t, in_=bv_bcast)
                for th in range(tha):
                    xt = xtp.tile([P, ck, thl], f32r, name=f"xt{th}", tag="xt")
                    for tb in range(thl // P):
                        gtb = th * (thl // P) + tb
                        xs_t = xsp.tile([P, c], f32)
                        nc.sync.dma_start(out=xs_t,
                                          in_=xkv[gtb * P:(gtb + 1) * P, :])
                        for cc in range(ck):
                            pt_ = tp_ps.tile([P, P], f32)
                            nc.tensor.transpose(pt_, xs_t[:, cc * P:(cc + 1) * P],
                                                ident32)
                            nc.vector.tensor_copy(
                                out=xt[:, cc, tb * P:(tb + 1) * P], in_=pt_)

                    for hsi in range(h // hs):
                        wk_t = wkp.tile([P, ck, hs], f32r, name=f"wk{th}_{hsi}",
                                        tag="wk")
                        nc.sync.dma_start(
                            out=wk_t,
                            in_=wk[:, hsi * hs:(hsi + 1) * hs]
                            .rearrange("(k p) h -> p k h", p=P).bitcast(f32r))
                        wv_t = wvp.tile([P, ck, hs], f32r, name=f"wv{th}_{hsi}",
                                        tag="wv")
                        nc.sync.dma_start(
                            out=wv_t,
                            in_=wv[:, hsi * hs:(hsi + 1) * hs]
                            .rearrange("(k p) h -> p k h", p=P).bitcast(f32r))
                        # kT rows for this strip
                        for h2 in range(hs // P):
                            hh = hsi * (hs // P) + h2
                            for tt in range(thl // ts):
                                ps = pr_ps.tile([P, ts], f32)
                                for cc in range(ck):
                                    nc.tensor.matmul(
                                        ps,
                                        lhsT=wk_t[:, cc, h2 * P:(h2 + 1) * P],
                                        rhs=xt[:, cc, tt * ts:(tt + 1) * ts],
                                        start=(cc == 0), stop=(cc == ck - 1))
                                nc.vector.tensor_scalar_add(
                                    out=kt[:, hh,
                                           th * thl + tt * ts:
                                           th * thl + (tt + 1) * ts],
                                    in0=ps, scalar1=bk_t[:, hh:hh + 1])
                        # v columns for this strip
                        vst = vstg.tile([P, thl // P, hs], bf16,
                                        name=f"vst{th}_{hsi}", tag="vst")
                        for sb in range(thl // P):
                            ps = pr_ps.tile([P, hs], f32, tag="vps")
                            for cc in range(ck):
                                nc.tensor.matmul(
                                    ps,
                                    lhsT=xt[:, cc, sb * P:(sb + 1) * P],
                                    rhs=wv_t[:, cc, :],
                                    start=(cc == 0), stop=(cc == ck - 1))
                            nc.vector.tensor_add(
                                out=vst[:, sb, :], in0=ps,
                                in1=bv_t[:, hsi * hs:(hsi + 1) * hs])
                        nc.sync.dma_start(
                            out=vs[th * (thl // P):(th + 1) * (thl // P), :,
                                   hsi * hs:(hsi + 1) * hs]
                            .rearrange("n p h -> p n h"),
                            in_=vst)

            # ---- phase B: qT ----
            tsq = min(512, tq)
            with (
                tc.tile_pool(name="xtqp", bufs=1) as xtqp,
                tc.tile_pool(name="xsp2", bufs=3) as xsp2,
                tc.tile_pool(name="wqp", bufs=2) as wqp,
                tc.tile_pool(name="tp_ps2", bufs=2, space="PSUM") as tp_ps2,
                tc.tile_pool(name="pr_ps2", bufs=4, space="PSUM") as pr_ps2,
            ):
                xtq = xtqp.tile([P, ck, tq], f32r)
                for tb in range(tq // P):
                    xs_t = xsp2.tile([P, c], f32)
                    nc.sync.dma_start(out=xs_t, in_=xq[tb * P:(tb + 1) * P, :])
                    for cc in range(ck):
                        pt_ = tp_ps2.tile([P, P], f32)
                        nc.tensor.transpose(pt_, xs_t[:, cc * P:(cc + 1) * P], ident32)
                        nc.vector.tensor_copy(out=xtq[:, cc, tb * P:(tb + 1) * P], in_=pt_)
                for hsi in range(h // hs):
                    wq_t = wqp.tile([P, ck, hs], f32r)
                    nc.sync.dma_start(
                        out=wq_t,
                        in_=wq[:, hsi * hs:(hsi + 1) * hs].rearrange("(k p) h -> p k h", p=P).bitcast(f32r))
                    for h2 in range(hs // P):
                        hh = hsi * (hs // P) + h2
                        for tt in range(tq // tsq):
                            ps = pr_ps2.tile([P, tsq], f32)
                            for cc in range(ck):
                                nc.tensor.matmul(
                                    ps,
                                    lhsT=wq_t[:, cc, h2 * P:(h2 + 1) * P],
                                    rhs=xtq[:, cc, tt * tsq:(tt + 1) * tsq],
                                    start=(cc == 0), stop=(cc == ck - 1))
                            nc.vector.tensor_scalar_add(
                                out=qt[:, hh, tt * tsq:(tt + 1) * tsq], in0=ps,
                                scalar1=bq_t[:, hh:hh + 1])

            # ---- phase C: attention ----
            with (
                tc.tile_pool(name="maskp", bufs=1) as maskp,
                tc.tile_pool(name="weip", bufs=2) as weip,
                tc.tile_pool(name="pp", bufs=2) as pp,
                tc.tile_pool(name="ptp", bufs=2) as ptp,
                tc.tile_pool(name="vinp", bufs=3) as vinp,
                tc.tile_pool(name="outp", bufs=2) as outp,
                tc.tile_pool(name="stats", bufs=8) as stats,
                tc.tile_pool(name="wei_ps", bufs=2, space="PSUM") as wei_ps,
                tc.tile_pool(name="pv_ps", bufs=1, space="PSUM") as pv_ps,
                tc.tile_pool(name="pt_ps", bufs=2, space="PSUM") as pt_ps,
            ):
                import concourse.mybir as mb
                mask_t = maskp.tile([P, ns, 2 * P], f32)
                nc.sync.dma_start(out=mask_t, in_=mask.rearrange("j p c -> p j c"))
                nstr = h // 512  # output column strips
                for j in range(ns):
                    scn = 2 * j + 2
                    s_end = scn * P
                    wt = weip.tile([P, t], f32)
                    for ss in range(math.ceil(s_end / 512)):
                        w = min(512, s_end - ss * 512)
                        ps = wei_ps.tile([P, 512], f32)
                        for hh in range(hk):
                            nc.tensor.matmul(
                                ps[:, :w],
                                lhsT=qt[:, hh, j * P:(j + 1) * P],
                                rhs=kt[:, hh, ss * 512:ss * 512 + w],
                                start=(hh == 0), stop=(hh == hk - 1))
                        nc.vector.tensor_copy(out=wt[:, ss * 512:ss * 512 + w],
                                              in_=ps[:, :w])
                    nc.vector.tensor_add(
                        out=wt[:, s_end - 2 * P:s_end],
                        in0=wt[:, s_end - 2 * P:s_end], in1=mask_t[:, j, :])
                    mneg = stats.tile([P, 1], f32)
                    nc.vector.tensor_reduce(
                        out=mneg, in_=wt[:, :s_end], axis=mb.AxisListType.X,
                        op=mb.AluOpType.max, negate=True)
                    ebias = stats.tile([P, 1], f32)
                    nc.vector.tensor_scalar_mul(ebias, mneg, scale)
                    pt_t = pp.tile([P, t], bf16)
                    rsum = stats.tile([P, 1], f32)
                    nc.scalar.activation(
                        out=pt_t[:, :s_end], in_=wt[:, :s_end],
                        func=mb.ActivationFunctionType.Exp,
                        bias=ebias, scale=scale, accum_out=rsum)
                    rinv = stats.tile([P, 1], f32)
                    nc.vector.reciprocal(rinv, rsum)
                    ptt = ptp.tile([P, nb, P], bf16)
                    for sc in range(scn):
                        pps = pt_ps.tile([P, P], bf16)
                        nc.tensor.transpose(pps, pt_t[:, sc * P:(sc + 1) * P], ident16)
                        nc.vector.tensor_copy(out=ptt[:, sc, :], in_=pps)
                    pv = [pv_ps.tile([P, 512], f32, tag=f"pv{n}", name=f"pv{n}_{j}")
                          for n in range(nstr)]
                    for sc in range(scn):
                        vt = vinp.tile([P, h], bf16)
                        nc.sync.dma_start(out=vt, in_=vs[sc, :, :])
                        for n in range(nstr):
                            nc.tensor.matmul(
                                pv[n], lhsT=ptt[:, sc, :],
                                rhs=vt[:, n * 512:(n + 1) * 512],
                                start=(sc == 0), stop=(sc == scn - 1))
                    ot = outp.tile([P, h], f32)
                    for n in range(nstr):
                        nc.scalar.activation(
                            out=ot[:, n * 512:(n + 1) * 512], in_=pv[n],
                            func=mb.ActivationFunctionType.Copy, scale=rinv)
                    nc.sync.dma_start(out=out[j * P:(j + 1) * P, :], in_=ot)

    nc.compile()
    return nc


class Runner:
    """Compiles the per-core program once and runs it on 8 cores via PJRT.

    Mirrors concourse.bass2jax.run_bass_via_pjrt's multi-core path, but
    keeps the jitted executable and device-resident inputs so repeated
    calls don't recompile or re-upload.
    """

    def __init__(self, t: int = T, c: int = C, h: int = H):
        import jax
        import concourse.mybir as mybir
        from concourse import bass2jax
        from jax.experimental.shard_map import shard_map
        from jax.sharding import Mesh, NamedSharding, PartitionSpec

        bass2jax.install_neuronx_cc_hook()
        self.jax = jax
        nc = build_nc(t, c, h)
        self.nc = nc
        self.n_cores = 8

        partition_name = (nc.partition_id_tensor.name
                          if nc.partition_id_tensor else None)
        in_names, out_names, out_avals, zero_outs = [], [], [], []
        for alloc in nc.m.functions[0].allocations:
            if not isinstance(alloc, mybir.MemoryLocationSet):
                continue
            name = alloc.memorylocations[0].name
            if alloc.kind == "ExternalInput":
                if name != partition_name:
                    in_names.append(name)
            elif alloc.kind == "ExternalOutput":
                shape = tuple(alloc.tensor_shape)
                dtype = mybir.dt.np(alloc.dtype)
                out_names.append(name)
                out_avals.append(jax.core.ShapedArray(shape, dtype))
                zero_outs.append(np.zeros(shape, dtype))
        self.in_names = list(in_names)
        self.out_names = out_names
        self.out_avals = out_avals
        n_params = len(in_names)
        all_in_names = in_names + out_names
        if partition_name is not None:
            all_in_names = all_in_names + [partition_name]

        def _body(*args):
            operands = list(args)
            if partition_name is not None:
                operands.append(bass2jax.partition_id_tensor())
            outs = bass2jax._bass_exec_p.bind(
                *operands,
                out_avals=tuple(out_avals),
                in_names=tuple(all_in_names),
                out_names=tuple(out_names),
                lowering_input_output_aliases=(),
                sim_require_finite=True,
                sim_require_nnan=True,
                nc=nc,
            )
            return tuple(outs)

        devices = jax.devices()[:self.n_cores]
        self.mesh = Mesh(np.asarray(devices), ("core",))
        nspec = (PartitionSpec("core"),) * (n_params + len(out_names))
        self._fn = jax.jit(
            shard_map(_body, mesh=self.mesh, in_specs=nspec,
                      out_specs=(PartitionSpec("core"),) * len(out_names),
                      check_rep=False),
            keep_unused=True)
        self._sharding = NamedSharding(self.mesh, PartitionSpec("core"))
        self._zero_outs = zero_outs

    def stage(self, in_maps: list[dict[str, np.ndarray]]):
        """Upload per-core inputs (list of 8 dicts) to the devices."""
        jax = self.jax
        args = []
        for name in self.in_names:
            cat = np.concatenate([np.asarray(m[name]) for m in in_maps], axis=0)
            args.append(jax.device_put(cat, self._sharding))
        for z in self._zero_outs:
            cat = np.zeros((self.n_cores * z.shape[0], *z.shape[1:]), z.dtype)
            args.append(jax.device_put(cat, self._sharding))
        return args

    def run_staged(self, args):
        return self._fn(*args)

    def __call__(self, in_maps: list[dict[str, np.ndarray]]):
        out_arrs = self.run_staged(self.stage(in_maps))
        self.jax.block_until_ready(out_arrs)
        return [
            {name: np.asarray(out_arrs[i]).reshape(
                self.n_cores, *self.out_avals[i].shape)[cid]
             for i, name in enumerate(self.out_names)}
            for cid in range(self.n_cores)
        ]


_runner_cache: dict = {}


def get_runner(t: int = T, c: int = C, h: int = H) -> Runner:
    key = (t, c, h)
    if key not in _runner_cache:
        _runner_cache[key] = Runner(t, c, h)
    return _runner_cache[key]


def make_in_maps(x, Wq, bq, Wk, bk, Wv, bv):
    """Build the 8 per-core input dicts from full inputs."""
    t = x.shape[1]
    nb = t // P
    blocks = half_blocks(nb)
    masks = make_masks(t)
    x = np.ascontiguousarray(np.asarray(x, dtype=np.float32))
    in_maps = []
    for core in range(8):
        b, half = divmod(core, 2)
        rows = np.concatenate(
            [np.arange(pb * P, (pb + 1) * P) for pb in sorted(blocks[half])])
        in_maps.append({
            "xkv": x[b],
            "xq": np.ascontiguousarray(x[b][rows]),
            "wq": np.asarray(Wq, np.float32), "wk": np.asarray(Wk, np.float32),
            "wv": np.asarray(Wv, np.float32),
            "bq": np.asarray(bq, np.float32), "bk": np.asarray(bk, np.float32),
            "bv": np.asarray(bv, np.float32),
            "mask": masks[half],
        })
    return in_maps


def assemble(results, t, h):
    """Scatter per-core [t/2, h] outputs back to [B, t, h]."""
    nb = t // P
    blocks = half_blocks(nb)
    out = np.empty((B, t, h), dtype=np.float32)
    for core in range(8):
        b, half = divmod(core, 2)
        rows = np.concatenate(
            [np.arange(pb * P, (pb + 1) * P) for pb in sorted(blocks[half])])
        out[b][rows] = results[core]["out"]
    return out


def kernel(x, Wq, bq, Wk, bk, Wv, bv):
    t, c, h = x.shape[1], x.shape[2], Wq.shape[1]
    runner = get_runner(t, c, h)
    results = runner(make_in_maps(x, Wq, bq, Wk, bk, Wv, bv))
    return assemble(results, t, h)
